# revision 34
# baseline (speedup 1.0000x reference)
"""8-core Trainium2 Bass kernel for nn_Attention_54778012893378.

Tensor-parallel over heads (2 heads/core), collective-free (v8). The
old AllToAll+wo tail cost ~54us of a 242us wall (each 512KB AllToAll
ran ~20us with a ~9us rendezvous floor, plus a 43us CC barrier and
warm-up collectives), so collectives were removed entirely: each core
computes a full-size PARTIAL output out_c = att_local @ wo[rows of its
2 heads] (same 1.07e9 MACs as a row-sliced wo after an AllToAll) and
DMAs an fp16 partial [2048, 2048] to HBM; the host sums the 8 partials
(the unshard step for a partial-sum layout). This kills the collective
tail, decouples the cores (no skew amplification), and lets the wo
block for chunk qc ride the next iteration's q-chain as pure-PE filler.

Datapath: fully 16-bit PE (fp16 x/weights/q/k, bf16 scores-exp/v/att/
wo), attention tiles emitted inside the projection dt-loops so the
ScalarE exp latency hides behind projection matmuls and the HAM clock
gate stays at 2.4 GHz; ~3.8us of throwaway matmuls warm the clock gate
during the initial input-DMA wait. Causal diagonal-band tiles restrict
scores/exp/PV to the valid column range (saves ~5us PE / ~8us exp).

Schedule per seq chunk sc (512 positions):
  x-chunk DMA for sc+1 (prefetched one iteration ahead); q-chain (both
  heads) with wo(sc-1) 512-col psum groups as filler from dt>=6;
  rope_q on DVE; remaining wo groups; k0/k1 chains carrying head-0/
  head-1 off-band attention tiles for chunk sc; rope_k0; v st-chains
  (psum->SBUF copies on ScalarE) with both heads' diagonal-band tiles;
  both heads drain in-iteration. Post-loop: only wo(qc3).
Scores are scaled via the per-query factor folded into q's rope tables
(single shared table when seq_scale is uniform); the causal mask is the
one pure-staircase 128-col block of a bf16 tile, applied as a multiply
after exp only on each band tile's partial block; the softmax
denominator accumulates in bf16 on DVE and broadcasts via one
ones-matmul, and the normalize is split per 128-row tile so wo can
start as soon as its slice lands.
"""

import numpy as np
import ml_dtypes

import concourse.bass as bass
import concourse.bacc as bacc
import concourse.tile as tile
import concourse.mybir as mybir
from concourse.bass_utils import run_bass_kernel_spmd

F32 = mybir.dt.float32
F32R = mybir.dt.float32r
F16 = mybir.dt.float16
BF16 = mybir.dt.bfloat16
AF = mybir.ActivationFunctionType
bf16 = ml_dtypes.bfloat16

# problem dims (hardcoded per spec)
S, D, H, HD, NC = 2048, 2048, 16, 128, 8
HL = H // NC            # local heads per core
CW = HL * HD            # per-core head-column width


def _rope_drain(nc, rtmp, ps, out_sl, cs, cs_w, w):
    """Full-width rope from psum [te;to]: oe = te*c - to*s ; oo = te*s + to*c.
    cs = [c;s] packed [128, w]; cs_w = [s;c] (swapped). Mixed psum+sbuf
    operands may differ in base partition; both-sbuf operands may not, so
    the upper halves are staged through base-0 copies."""
    u1 = rtmp.tile([128, w], F16, tag="u1", name="u1")
    u2 = rtmp.tile([128, w], F16, tag="u2", name="u2")
    nc.vector.tensor_mul(u1, ps, cs)         # [te*c ; to*s]
    nc.vector.tensor_mul(u2, ps, cs_w)       # [te*s ; to*c] -- frees psum
    b1 = rtmp.tile([64, w], F16, tag="b1", name="b1")
    b2 = rtmp.tile([64, w], F16, tag="b2", name="b2")
    nc.vector.tensor_copy(b1, u1[64:128, :])
    nc.vector.tensor_copy(b2, u2[64:128, :])
    nc.vector.tensor_sub(out_sl[0:64, :], u1[0:64, :], b1)
    nc.vector.tensor_add(out_sl[64:128, :], u2[0:64, :], b2)


def build_nc(causal, shared, s=S, d=D, qc_w=512):
    assert HL == 2
    ndt = d // 128          # contraction tiles over model dim
    nkt = s // 128          # kpos tiles
    nsc = s // qc_w         # seq/q chunks
    nst = qc_w // 128       # kpos tiles per q-chunk band
    sc_w = qc_w

    nc = bacc.Bacc("TRN2", target_bir_lowering=False, debug=False, num_devices=NC)

    xt = nc.dram_tensor("xt", [d, s], F16, kind="ExternalInput").ap()
    wqkv = nc.dram_tensor("wqkv", [d, 3 * CW], F16, kind="ExternalInput").ap()
    # only the rows of wo that multiply this core's heads: [CW, d]
    wo = nc.dram_tensor("wo", [CW, d], BF16, kind="ExternalInput").ap()
    cq = nc.dram_tensor("cq", [128, s], F16, kind="ExternalInput").ap()
    if not shared:
        sq = nc.dram_tensor("sq", [128, s], F16, kind="ExternalInput").ap()
    cksk = nc.dram_tensor("cksk", [128, s], F16, kind="ExternalInput").ap()
    if causal:
        # staircase: diagonal pattern m is emb[:, 512-128m : 1024-128m]
        emb = nc.dram_tensor("em", [128, qc_w + nst * 128], BF16,
                             kind="ExternalInput").ap()
    else:
        emb = nc.dram_tensor("em", [s, s], BF16, kind="ExternalInput").ap()
    # full-size PARTIAL output (this core's 2 heads through wo); host sums
    out = nc.dram_tensor("out", [s, d], F16, kind="ExternalOutput").ap()

    import contextlib

    with tile.TileContext(nc, num_cores=NC) as tc:
        with contextlib.ExitStack() as top:
            qkv = top.enter_context(tc.tile_pool(name="qkv", bufs=1))
            qT_s = qkv.tile([128, HL, s], F16)
            kT_s = qkv.tile([128, HL, s], F16)
            v_s = qkv.tile([128, nkt, CW], BF16)
            att_s = qkv.tile([128, HL, s], BF16)
            ones_s = qkv.tile([128, 128], BF16)
            nc.vector.memset(ones_s, 1.0)
            warm_s = qkv.tile([128, 512], BF16)
            nc.vector.memset(warm_s, 1.0)

            consts = top.enter_context(tc.tile_pool(name="consts", bufs=1))
            cq_s = consts.tile([128, s], F16)     # q [c;s] (scaled)
            cqw_s = consts.tile([128, s], F16)    # swapped [s;c]
            if not shared:
                sq_s = consts.tile([128, s], F16)
                sqw_s = consts.tile([128, s], F16)
            else:
                sq_s, sqw_s = cq_s, cqw_s
            ck_s = consts.tile([128, s], F16)     # k [c;s]
            ckw_s = consts.tile([128, s], F16)
            wqkv_sb = consts.tile([128, ndt, 3 * CW], F16)
            em_s = None
            if causal:
                em_s = consts.tile([128, qc_w + nst * 128], BF16)

            wop = top.enter_context(tc.tile_pool(name="wop", bufs=1))
            wo_sb = wop.tile([128, HL, d], BF16)
            wo_r = wo.rearrange("(h p) n -> h p n", p=128)

            xch = top.enter_context(tc.tile_pool(name="xch", bufs=16))
            rtmp = top.enter_context(tc.tile_pool(name="rtmp", bufs=1))
            ep = top.enter_context(tc.tile_pool(name="ep", bufs=4))
            rbp = top.enter_context(tc.tile_pool(name="rbp", bufs=2))
            accp = top.enter_context(tc.tile_pool(name="accp", bufs=2))
            outp = top.enter_context(tc.tile_pool(name="outp", bufs=2))
            emp = None
            if not causal:
                emp = top.enter_context(tc.tile_pool(name="emp", bufs=4))

            psqk = top.enter_context(tc.tile_pool(name="psqk", bufs=2, space="PSUM"))
            psv = top.enter_context(tc.tile_pool(name="psv", bufs=2, space="PSUM"))
            pss = top.enter_context(tc.tile_pool(name="pss", bufs=2, space="PSUM"))
            pso = top.enter_context(tc.tile_pool(name="pso", bufs=2, space="PSUM"))

            wqkv_p = wqkv.rearrange("(dt p) c -> p dt c", p=128)
            xt_p = xt.rearrange("(dt p) z -> p dt z", p=128)

            def attn_units(h, qc):
                """Generator: one attention tile per step; PV deferred one
                tile so exp latency hides behind the next scores matmul.
                Normalized output lands in att_s[:, h, chunk]."""
                qsl = slice(qc * qc_w, (qc + 1) * qc_w)
                n_kt = nst * (qc + 1) if causal else nkt
                o_ps = pso.tile([128, qc_w], F32, tag="pso", name=f"ops{h}_{qc}")
                acc = accp.tile([128, qc_w], BF16, tag="acc", name=f"acc{h}_{qc}")
                prev = None
                for kt in range(n_kt):
                    # diagonal-band tiles: columns below the staircase are
                    # all-masked, so restrict scores/exp/PV to the valid
                    # [cb:] slice (the partial 128-wide block keeps the pure
                    # staircase mask at em_s[:, qc_w:qc_w+128])
                    m = kt - nst * qc
                    cb = 128 * m if (causal and m > 0) else 0
                    s_ps = pss.tile([128, qc_w], F32, tag="pss", name=f"sps{h}_{qc}_{kt}")
                    nc.tensor.matmul(
                        s_ps[:, cb:],
                        lhsT=kT_s[:, h, kt * 128 : (kt + 1) * 128],
                        rhs=qT_s[:, h, qc * qc_w + cb : (qc + 1) * qc_w],
                        start=True,
                        stop=True,
                    )
                    if prev is not None:
                        pe_t, pkt, pcb = prev
                        nc.tensor.matmul(
                            o_ps[:, pcb:],
                            lhsT=v_s[:, pkt, HD * h : HD * (h + 1)],
                            rhs=pe_t[:, pcb:],
                            start=(pkt == 0),
                            stop=False,
                        )
                    e = ep.tile([128, qc_w], BF16, tag="e", name=f"e{h}_{qc}_{kt}")
                    nc.scalar.activation(e[:, cb:], s_ps[:, cb:], AF.Exp)
                    if causal:
                        if m >= 0:
                            nc.vector.tensor_mul(
                                e[:, cb : cb + 128],
                                e[:, cb : cb + 128],
                                em_s[:, qc_w : qc_w + 128],
                            )
                    else:
                        emt = emp.tile([128, qc_w], BF16, tag="em", name=f"emt{h}_{qc}_{kt}")
                        nc.sync.dma_start(emt, emb[kt * 128 : (kt + 1) * 128, qsl])
                        nc.vector.tensor_mul(e, e, emt)
                    if kt == 0:
                        nc.vector.tensor_copy(acc, e)
                    else:
                        nc.vector.tensor_add(acc[:, cb:], acc[:, cb:], e[:, cb:])
                    prev = (e, kt, cb)
                    yield
                pe_t, pkt, pcb = prev
                nc.tensor.matmul(
                    o_ps[:, pcb:],
                    lhsT=v_s[:, pkt, HD * h : HD * (h + 1)],
                    rhs=pe_t[:, pcb:],
                    start=(pkt == 0),
                    stop=True,
                )
                # denominator, pre-broadcast across partitions by a [128,128]
                # ones stationary against the bf16 accumulator
                d_ps = pss.tile([128, qc_w], F32, tag="pss", name=f"dps{h}_{qc}")
                nc.tensor.matmul(d_ps, lhsT=ones_s, rhs=acc, start=True, stop=True)
                rec = rbp.tile([128, qc_w], F32, tag="rb", name=f"rb{h}_{qc}")
                nc.vector.reciprocal_approx_fast(rec, d_ps)
                # normalize per 128-wide row-tile so the wo matmul for tile
                # rt can start as soon as slice rt is written
                for i in range(qc_w // 128):
                    isl = slice(i * 128, (i + 1) * 128)
                    nc.vector.tensor_mul(
                        att_s[:, h, qc * qc_w + i * 128 : qc * qc_w + (i + 1) * 128],
                        o_ps[:, isl], rec[:, isl])
                yield

            def step(g):
                if g is None:
                    return False
                try:
                    next(g)
                    return True
                except StopIteration:
                    return False

            def drain(g):
                while step(g):
                    pass

            def wo_units(qc, rts, pools=None, fine=False):
                """Partial-output block for chunk qc, row-tiles rts: for each
                128-row tile, out[rows, :] = sum_h att_s[:,h,rows].T @ wo_sb[h].
                Pure PE + psum-copy; pss+pso (free whenever no attention
                generator is live) give a 4-bank rotation so the copies
                hide. Yields after each 512-col group."""
                if pools is None:
                    pools = (pss, pso)
                base = qc * qc_w
                for rt in rts:
                    rsl = slice(base + rt * 128, base + (rt + 1) * 128)
                    ost = outp.tile([128, d], F16, tag="osb", name=f"osb{qc}_{rt}")
                    for nk in range(d // 512):
                        nsl = slice(nk * 512, (nk + 1) * 512)
                        pool = pools[nk % len(pools)]
                        w_ps = pool.tile([128, 512], F32, tag=pool.name,
                                         name=f"wps{qc}_{rt}_{nk}")
                        for hh in range(HL):
                            nc.tensor.matmul(
                                w_ps,
                                lhsT=att_s[:, hh, rsl],
                                rhs=wo_sb[:, hh, nsl],
                                start=(hh == 0),
                                stop=(hh == HL - 1),
                            )
                        if fine and rt == rts[-1]:
                            # final tile: quarter-width copies split across
                            # both engines shorten the last serial chain
                            for qq in range(2):
                                qsl2 = slice(nk * 512 + qq * 256,
                                             nk * 512 + (qq + 1) * 256)
                                if qq == 0:
                                    nc.vector.tensor_copy(ost[:, qsl2], w_ps[:, qq * 256 : (qq + 1) * 256])
                                else:
                                    nc.scalar.activation(ost[:, qsl2], w_ps[:, qq * 256 : (qq + 1) * 256], AF.Copy)
                        elif nk % 2 == 0:
                            nc.vector.tensor_copy(ost[:, nsl], w_ps)
                        else:
                            nc.scalar.activation(ost[:, nsl], w_ps, AF.Copy)
                        if fine and rt == rts[-1]:
                            # last tile of the kernel: flush per 512 so the
                            # final HBM write is as small as possible
                            nc.sync.dma_start(out[rsl, nsl], ost[:, nsl])
                        elif nk % 2 == 1:
                            # flush per 1024-col half so the final HBM write
                            # drains behind the remaining compute
                            hsl = slice((nk - 1) * 512, (nk + 1) * 512)
                            nc.sync.dma_start(out[rsl, hsl], ost[:, hsl])
                        yield

            def wo_block(qc, rts=None, fine=False):
                drain(wo_units(qc, rts if rts is not None else range(nst),
                               fine=fine))

            def issue_x(sc2, with_w=False):
                lst = []
                for dp in range(ndt // 2):
                    if with_w:
                        nc.sync.dma_start(
                            wqkv_sb[:, 2 * dp : 2 * dp + 2, :],
                            wqkv_p[:, 2 * dp : 2 * dp + 2, :],
                        )
                    t = xch.tile([128, 2, sc_w], F16, tag="xch", name=f"xch{sc2}_{dp}")
                    nc.sync.dma_start(
                        t, xt_p[:, 2 * dp : 2 * dp + 2,
                                sc2 * sc_w : (sc2 + 1) * sc_w])
                    lst.append(t)
                return lst

            # HAM warm-up: ~3.8us of throwaway matmuls during the initial
            # input-DMA wait, so the clock gate is already at 2.4 GHz when
            # the first real matmul's operands land (cold MMs run at 1.2)
            warm_ps = pso.tile([128, 512], F32, tag="pso", name="warm_ps")
            for i in range(14):
                nc.tensor.matmul(warm_ps, lhsT=ones_s, rhs=warm_s,
                                 start=(i == 0), stop=(i == 13))

            # ---------------- main loop with interleaved attention ----------------
            xnext = None
            for sc in range(nsc):
                scs = slice(sc * sc_w, (sc + 1) * sc_w)
                if sc == 0:
                    xps = issue_x(0, with_w=True)
                    # rope tables + mask + next x chunk + wo, ordered by
                    # first-use time so nothing steals early HBM bandwidth
                    # from the sc0 x/weight stream
                    nc.sync.dma_start(cq_s, cq)
                    if not shared:
                        nc.sync.dma_start(sq_s, sq)
                    nc.sync.dma_start(ck_s, cksk)
                    if causal:
                        nc.sync.dma_start(em_s, emb)
                    xnext = issue_x(1)
                    for hh in range(HL):
                        nc.sync.dma_start(wo_sb[:, hh, :], wo_r[hh])
                    pairs = [(cq_s, cqw_s), (ck_s, ckw_s)]
                    if not shared:
                        pairs.insert(1, (sq_s, sqw_s))
                    for src_t, dst_t in pairs:
                        nc.vector.tensor_copy(dst_t[0:64, :], src_t[64:128, :])
                        nc.vector.tensor_copy(dst_t[64:128, :], src_t[0:64, :])
                else:
                    xps = xnext
                    if sc + 1 < nsc:
                        xnext = issue_x(sc + 1)
                xts = [xps[dt // 2][:, dt % 2, :] for dt in range(ndt)]

                # wo for chunk sc-1 (both heads' attention drained at the end
                # of iteration sc-1) rides this sc's q-chain as pure-PE
                # filler; its psum groups use the pss/pso banks the previous
                # iteration's generators have released by the time the ring
                # comes around
                gwo = wo_units(sc - 1, range(nst)) if (causal and sc >= 1) else None
                q_ps = [psqk.tile([128, sc_w], F32, tag="psqk", name=f"qps{sc}_{_h}")
                        for _h in range(HL)]
                # sc=0 is DMA-bound: fold the k chains into the same per-dt
                # loop (borrowing the pss banks, free before the first
                # attention unit) so the PE does 4 MMs per arriving tile
                # pair instead of 2 and the post-DMA phase starts earlier
                k_pss = None
                if sc == 0:
                    k_pss = [pss.tile([128, sc_w], F32, tag="pss", name=f"kps0_{_h}")
                             for _h in range(HL)]
                for dt in range(ndt):
                    fl = dict(start=(dt == 0), stop=(dt == ndt - 1))
                    for h in range(HL):
                        nc.tensor.matmul(
                            q_ps[h],
                            lhsT=wqkv_sb[:, dt, HD * h : HD * (h + 1)],
                            rhs=xts[dt],
                            **fl,
                        )
                    if k_pss is not None:
                        for h in range(HL):
                            nc.tensor.matmul(
                                k_pss[h],
                                lhsT=wqkv_sb[:, dt, CW + HD * h : CW + HD * (h + 1)],
                                rhs=xts[dt],
                                **fl,
                            )
                    # start the wo filler a few dt-steps in: its psum ring
                    # slots free up only once the previous iteration's
                    # finalization chain clears the DVE queue
                    if dt >= 6:
                        step(gwo)
                _rope_drain(nc, rtmp, q_ps[0], qT_s[:, 0, scs],
                            cq_s[:, scs], cqw_s[:, scs], sc_w)
                _rope_drain(nc, rtmp, q_ps[1], qT_s[:, 1, scs],
                            sq_s[:, scs], sqw_s[:, scs], sc_w)
                drain(gwo)

                # BOTH heads' attention for THIS q chunk: head-h off-band
                # tiles ride the k-h chain, diagonal-band tiles ride the v
                # chains; everything drains inside this iteration
                gens = [attn_units(h, sc) if causal else None for h in range(HL)]
                off_band = nst * sc
                emitted = [0, 0]
                # k0 chain, then k1 chain (each reuses the psum slot of the
                # same-index q_ps, freed by its rope -- the wo filler above
                # guarantees the ropes are long done, so no PE stall)
                k1_ps = None
                if k_pss is not None:
                    _rope_drain(nc, rtmp, k_pss[0], kT_s[:, 0, scs],
                                ck_s[:, scs], ckw_s[:, scs], sc_w)
                    k1_ps = k_pss[1]
                else:
                    for h in range(HL):
                        k_ps = psqk.tile([128, sc_w], F32, tag="psqk",
                                         name=f"kps{sc}_{h}")
                        for dt in range(ndt):
                            nc.tensor.matmul(
                                k_ps,
                                lhsT=wqkv_sb[:, dt, CW + HD * h : CW + HD * (h + 1)],
                                rhs=xts[dt],
                                start=(dt == 0),
                                stop=(dt == ndt - 1),
                            )
                            if dt >= 2 and emitted[h] < off_band:
                                if step(gens[h]):
                                    emitted[h] += 1
                        if h == 0:
                            _rope_drain(nc, rtmp, k_ps, kT_s[:, 0, scs],
                                        ck_s[:, scs], ckw_s[:, scs], sc_w)
                        else:
                            k1_ps = k_ps
                # v chains; rope_k1 + v copies interleave on DVE; both heads'
                # diagonal-band attention tiles follow their v copies
                for st in range(nst):
                    v_ps = psv.tile([128, CW], F32, tag="psv", name=f"vps{sc}_{st}")
                    for dt in range(ndt):
                        nc.tensor.matmul(
                            v_ps,
                            lhsT=xts[dt][:, st * 128 : (st + 1) * 128],
                            rhs=wqkv_sb[:, dt, 2 * CW : 3 * CW],
                            start=(dt == 0),
                            stop=(dt == ndt - 1),
                        )
                    nc.scalar.activation(v_s[:, sc * nst + st, :], v_ps, AF.Copy)
                    if st == 0:
                        _rope_drain(nc, rtmp, k1_ps, kT_s[:, 1, scs],
                                    ck_s[:, scs], ckw_s[:, scs], sc_w)
                    if st >= 1:
                        step(gens[0])
                        step(gens[1])
                drain(gens[0])
                drain(gens[1])

            if causal:
                wo_block(nsc - 1, fine=True)
            else:
                for qc in range(nsc):
                    drain(attn_units(0, qc))
                    drain(attn_units(1, qc))
                    wo_block(qc, fine=(qc == nsc - 1))

    nc.compile()
    return nc


def host_prep(inputs, s=S, d=D, qc_w=512):
    f16 = np.float16
    x = np.ascontiguousarray(np.asarray(inputs["x"], dtype=np.float32)[0])
    wq = np.asarray(inputs["wq"], dtype=np.float32)
    wk = np.asarray(inputs["wk"], dtype=np.float32)
    wv = np.asarray(inputs["wv"], dtype=np.float32)
    wo = np.asarray(inputs["wo"], dtype=np.float32)
    ss = np.asarray(inputs["seq_scale"], dtype=np.float32).reshape(H)
    cos = np.asarray(inputs["freqs_cos"], dtype=np.float32)
    sin = np.asarray(inputs["freqs_sin"], dtype=np.float32)
    mask = np.asarray(inputs["mask"], dtype=np.float32)[0, 0]
    sll = np.asarray(inputs["section_log_len"], dtype=np.float32).reshape(s)

    nst = qc_w // 128
    zero = mask == 0.0
    causal = bool(
        np.array_equal(zero, np.tril(np.ones((s, s), bool)))
        and np.all(mask[~zero] <= -1e8)
    )
    shared = bool(np.all(ss == ss[0]))

    if causal:
        # staircase base [128, qc_w + nst*128]: diagonal pattern m (kpos
        # tile kt = nst*qc + m vs q chunk qc) is emb[:, qc_w-128m :
        # 2*qc_w-128m], i.e. emb[dk, c] = 1 iff (dk + 128m) <= dq with
        # dq = c - (qc_w - 128m)  <=>  dk <= c - qc_w
        w_em = qc_w + nst * 128
        ccol = np.arange(w_em)[None, :]
        crow = np.arange(128)[:, None]
        em_in = np.ascontiguousarray(
            (crow <= ccol - qc_w).astype(np.float32).astype(bf16)
        )
    else:
        em_in = np.ascontiguousarray(np.exp(np.minimum(mask, 0.0)).T.astype(bf16))

    perm = np.concatenate([np.arange(0, HD, 2), np.arange(1, HD, 2)])
    xt = np.ascontiguousarray(x.T.astype(f16))
    scale = sll / np.sqrt(HD)
    cksk = np.ascontiguousarray(np.concatenate([cos.T, sin.T], axis=0).astype(f16))

    in_maps = []
    for i in range(NC):
        wq_s = np.concatenate(
            [wq[:, CW * i + HD * h : CW * i + HD * (h + 1)][:, perm] for h in range(HL)],
            axis=1,
        )
        wk_s = np.concatenate(
            [wk[:, CW * i + HD * h : CW * i + HD * (h + 1)][:, perm] for h in range(HL)],
            axis=1,
        )
        wv_s = wv[:, CW * i : CW * (i + 1)]
        # per-head packed [cos; sin] scaled tables (one shared table when
        # seq_scale is uniform)
        cqt = np.concatenate(
            [cos.T * (scale * ss[HL * i])[None, :],
             sin.T * (scale * ss[HL * i])[None, :]], axis=0
        )
        m = {
            "xt": xt,
            "wqkv": np.ascontiguousarray(
                np.concatenate([wq_s, wk_s, wv_s], axis=1).astype(f16)
            ),
            "wo": np.ascontiguousarray(wo[CW * i : CW * (i + 1)].astype(bf16)),
            "cq": np.ascontiguousarray(cqt.astype(f16)),
            "cksk": cksk,
            "em": em_in,
        }
        if not shared:
            sqt = np.concatenate(
                [cos.T * (scale * ss[HL * i + 1])[None, :],
                 sin.T * (scale * ss[HL * i + 1])[None, :]], axis=0
            )
            m["sq"] = np.ascontiguousarray(sqt.astype(f16))
        in_maps.append(m)
    return in_maps, causal, shared


_NC_CACHE = {}


def _get_nc(causal, shared):
    key = (causal, shared)
    if key not in _NC_CACHE:
        _NC_CACHE[key] = build_nc(causal, shared)
    return _NC_CACHE[key]


def gather(res):
    """Sum the 8 fp16 partial outputs (unshard for a partial-sum layout)."""
    acc = res.results[0]["out"].astype(np.float32)
    for i in range(1, NC):
        acc += res.results[i]["out"].astype(np.float32)
    return acc[None]


def kernel(**inputs) -> np.ndarray:
    in_maps, causal, shared = host_prep(inputs)
    nc = _get_nc(causal, shared)
    res = run_bass_kernel_spmd(nc, in_maps, core_ids=list(range(NC)))
    return gather(res)


# revision 36
# speedup vs baseline: 1.1764x; 1.1764x over previous
"""8-core Trainium2 Bass kernel for nn_Attention_54778012893378.

Tensor-parallel over heads (2 heads/core), collective-free. The
old AllToAll+wo tail cost ~54us of a 242us wall (each 512KB AllToAll
ran ~20us with a ~9us rendezvous floor, plus a 43us CC barrier and
warm-up collectives), so collectives were removed entirely: each core
computes a full-size PARTIAL output out_c = att_local @ wo[rows of its
2 heads] (same 1.07e9 MACs as a row-sliced wo after an AllToAll) and
DMAs an fp16 partial [2048, 2048] to HBM; the host sums the 8 partials
(the unshard step for a partial-sum layout). This kills the collective
tail, decouples the cores (no skew amplification), and lets the wo
block for chunk qc ride the next iteration's q-chain as pure-PE filler.

Datapath: fully 16-bit PE (fp16 x/weights/q/k, bf16 scores-exp/v/att/
wo), attention tiles emitted inside the projection dt-loops so the
ScalarE exp latency hides behind projection matmuls and the HAM clock
gate stays at 2.4 GHz; ~3.8us of throwaway matmuls warm the clock gate
during the initial input-DMA wait. Causal diagonal-band tiles restrict
scores/exp/PV to the valid column range (saves ~5us PE / ~8us exp).

Schedule per seq chunk sc (512 positions):
  x-chunk DMA for sc+1 (prefetched one iteration ahead); q-chain (both
  heads) with wo(sc-1) 512-col psum groups as filler from dt>=6;
  rope_q on DVE; remaining wo groups; k0/k1 chains carrying head-0/
  head-1 off-band attention tiles for chunk sc; rope_k0; v st-chains
  (psum->SBUF copies on ScalarE) with both heads' diagonal-band tiles;
  both heads drain in-iteration. Post-loop: only wo(qc3). sc=0 is
  DMA-bound, so its k chains fold into the per-dt q loop (4 MMs per
  arriving tile pair) to fill the input-stream window.
Scores are scaled via the per-query factor folded into q's rope tables
(single shared table when seq_scale is uniform); the causal mask is the
one pure-staircase 128-col block of a bf16 tile, applied as a multiply
after exp only on each band tile's partial block; the softmax
denominator accumulates in bf16 on DVE and broadcasts via one
ones-matmul, and the normalize is split per 128-row tile so wo can
start as soon as its slice lands.
"""

import numpy as np
import ml_dtypes

import concourse.bass as bass
import concourse.bacc as bacc
import concourse.tile as tile
import concourse.mybir as mybir
from concourse.bass_utils import run_bass_kernel_spmd

F32 = mybir.dt.float32
F32R = mybir.dt.float32r
F16 = mybir.dt.float16
BF16 = mybir.dt.bfloat16
AF = mybir.ActivationFunctionType
bf16 = ml_dtypes.bfloat16

# problem dims (hardcoded per spec)
S, D, H, HD, NC = 2048, 2048, 16, 128, 8
HL = H // NC            # local heads per core
CW = HL * HD            # per-core head-column width


def _rope_drain(nc, rtmp, ps, out_sl, cs, cs_w, w):
    """Full-width rope from psum [te;to]: oe = te*c - to*s ; oo = te*s + to*c.
    cs = [c;s] packed [128, w]; cs_w = [s;c] (swapped). Mixed psum+sbuf
    operands may differ in base partition; both-sbuf operands may not, so
    the upper halves are staged through base-0 copies."""
    u1 = rtmp.tile([128, w], F16, tag="u1", name="u1")
    u2 = rtmp.tile([128, w], F16, tag="u2", name="u2")
    nc.vector.tensor_mul(u1, ps, cs)         # [te*c ; to*s]
    nc.vector.tensor_mul(u2, ps, cs_w)       # [te*s ; to*c] -- frees psum
    b1 = rtmp.tile([64, w], F16, tag="b1", name="b1")
    b2 = rtmp.tile([64, w], F16, tag="b2", name="b2")
    nc.vector.tensor_copy(b1, u1[64:128, :])
    nc.vector.tensor_copy(b2, u2[64:128, :])
    nc.vector.tensor_sub(out_sl[0:64, :], u1[0:64, :], b1)
    nc.vector.tensor_add(out_sl[64:128, :], u2[0:64, :], b2)


def build_nc(causal, shared, s=S, d=D, qc_w=512):
    assert HL == 2
    ndt = d // 128          # contraction tiles over model dim
    nkt = s // 128          # kpos tiles
    nsc = s // qc_w         # seq/q chunks
    nst = qc_w // 128       # kpos tiles per q-chunk band
    sc_w = qc_w

    nc = bacc.Bacc("TRN2", target_bir_lowering=False, debug=False, num_devices=NC)

    xt = nc.dram_tensor("xt", [d, s], F16, kind="ExternalInput").ap()
    wqkv = nc.dram_tensor("wqkv", [d, 3 * CW], F16, kind="ExternalInput").ap()
    # only the rows of wo that multiply this core's heads: [CW, d]
    wo = nc.dram_tensor("wo", [CW, d], BF16, kind="ExternalInput").ap()
    cq = nc.dram_tensor("cq", [128, s], F16, kind="ExternalInput").ap()
    if not shared:
        sq = nc.dram_tensor("sq", [128, s], F16, kind="ExternalInput").ap()
    cksk = nc.dram_tensor("cksk", [128, s], F16, kind="ExternalInput").ap()
    if causal:
        # staircase: diagonal pattern m is emb[:, 512-128m : 1024-128m]
        emb = nc.dram_tensor("em", [128, qc_w + nst * 128], BF16,
                             kind="ExternalInput").ap()
    else:
        emb = nc.dram_tensor("em", [s, s], BF16, kind="ExternalInput").ap()
    # full-size PARTIAL output (this core's 2 heads through wo); host sums
    out = nc.dram_tensor("out", [s, d], F16, kind="ExternalOutput").ap()

    import contextlib

    with tile.TileContext(nc, num_cores=NC) as tc:
        with contextlib.ExitStack() as top:
            qkv = top.enter_context(tc.tile_pool(name="qkv", bufs=1))
            qT_s = qkv.tile([128, HL, s], F16)
            kT_s = qkv.tile([128, HL, s], F16)
            v_s = qkv.tile([128, nkt, CW], BF16)
            att_s = qkv.tile([128, HL, s], BF16)
            ones_s = qkv.tile([128, 128], BF16)
            nc.vector.memset(ones_s, 1.0)
            warm_s = qkv.tile([128, 512], BF16)
            nc.vector.memset(warm_s, 1.0)

            consts = top.enter_context(tc.tile_pool(name="consts", bufs=1))
            cq_s = consts.tile([128, s], F16)     # q [c;s] (scaled)
            cqw_s = consts.tile([128, s], F16)    # swapped [s;c]
            if not shared:
                sq_s = consts.tile([128, s], F16)
                sqw_s = consts.tile([128, s], F16)
            else:
                sq_s, sqw_s = cq_s, cqw_s
            ck_s = consts.tile([128, s], F16)     # k [c;s]
            ckw_s = consts.tile([128, s], F16)
            wqkv_sb = consts.tile([128, ndt, 3 * CW], F16)
            em_s = None
            if causal:
                em_s = consts.tile([128, qc_w + nst * 128], BF16)

            wop = top.enter_context(tc.tile_pool(name="wop", bufs=1))
            wo_sb = wop.tile([128, HL, d], BF16)
            wo_r = wo.rearrange("(h p) n -> h p n", p=128)

            xch = top.enter_context(tc.tile_pool(name="xch", bufs=16))
            rtmp = top.enter_context(tc.tile_pool(name="rtmp", bufs=1))
            ep = top.enter_context(tc.tile_pool(name="ep", bufs=4))
            rbp = top.enter_context(tc.tile_pool(name="rbp", bufs=2))
            accp = top.enter_context(tc.tile_pool(name="accp", bufs=2))
            outp = top.enter_context(tc.tile_pool(name="outp", bufs=2))
            emp = None
            if not causal:
                emp = top.enter_context(tc.tile_pool(name="emp", bufs=4))

            psqk = top.enter_context(tc.tile_pool(name="psqk", bufs=2, space="PSUM"))
            psv = top.enter_context(tc.tile_pool(name="psv", bufs=2, space="PSUM"))
            pss = top.enter_context(tc.tile_pool(name="pss", bufs=2, space="PSUM"))
            pso = top.enter_context(tc.tile_pool(name="pso", bufs=2, space="PSUM"))

            wqkv_p = wqkv.rearrange("(dt p) c -> p dt c", p=128)
            xt_p = xt.rearrange("(dt p) z -> p dt z", p=128)

            def attn_units(h, qc):
                """Generator: one attention tile per step; PV deferred one
                tile so exp latency hides behind the next scores matmul.
                Normalized output lands in att_s[:, h, chunk]."""
                qsl = slice(qc * qc_w, (qc + 1) * qc_w)
                n_kt = nst * (qc + 1) if causal else nkt
                o_ps = pso.tile([128, qc_w], F32, tag="pso", name=f"ops{h}_{qc}")
                acc = accp.tile([128, qc_w], BF16, tag="acc", name=f"acc{h}_{qc}")
                prev = None
                for kt in range(n_kt):
                    # diagonal-band tiles: columns below the staircase are
                    # all-masked, so restrict scores/exp/PV to the valid
                    # [cb:] slice (the partial 128-wide block keeps the pure
                    # staircase mask at em_s[:, qc_w:qc_w+128])
                    m = kt - nst * qc
                    cb = 128 * m if (causal and m > 0) else 0
                    s_ps = pss.tile([128, qc_w], F32, tag="pss", name=f"sps{h}_{qc}_{kt}")
                    nc.tensor.matmul(
                        s_ps[:, cb:],
                        lhsT=kT_s[:, h, kt * 128 : (kt + 1) * 128],
                        rhs=qT_s[:, h, qc * qc_w + cb : (qc + 1) * qc_w],
                        start=True,
                        stop=True,
                    )
                    if prev is not None:
                        pe_t, pkt, pcb = prev
                        nc.tensor.matmul(
                            o_ps[:, pcb:],
                            lhsT=v_s[:, pkt, HD * h : HD * (h + 1)],
                            rhs=pe_t[:, pcb:],
                            start=(pkt == 0),
                            stop=False,
                        )
                    e = ep.tile([128, qc_w], BF16, tag="e", name=f"e{h}_{qc}_{kt}")
                    nc.scalar.activation(e[:, cb:], s_ps[:, cb:], AF.Exp)
                    if causal:
                        if m >= 0:
                            nc.vector.tensor_mul(
                                e[:, cb : cb + 128],
                                e[:, cb : cb + 128],
                                em_s[:, qc_w : qc_w + 128],
                            )
                    else:
                        emt = emp.tile([128, qc_w], BF16, tag="em", name=f"emt{h}_{qc}_{kt}")
                        nc.sync.dma_start(emt, emb[kt * 128 : (kt + 1) * 128, qsl])
                        nc.vector.tensor_mul(e, e, emt)
                    if kt == 0:
                        nc.vector.tensor_copy(acc, e)
                    else:
                        nc.vector.tensor_add(acc[:, cb:], acc[:, cb:], e[:, cb:])
                    prev = (e, kt, cb)
                    yield
                pe_t, pkt, pcb = prev
                nc.tensor.matmul(
                    o_ps[:, pcb:],
                    lhsT=v_s[:, pkt, HD * h : HD * (h + 1)],
                    rhs=pe_t[:, pcb:],
                    start=(pkt == 0),
                    stop=True,
                )
                # denominator, pre-broadcast across partitions by a [128,128]
                # ones stationary against the bf16 accumulator
                d_ps = pss.tile([128, qc_w], F32, tag="pss", name=f"dps{h}_{qc}")
                nc.tensor.matmul(d_ps, lhsT=ones_s, rhs=acc, start=True, stop=True)
                rec = rbp.tile([128, qc_w], F32, tag="rb", name=f"rb{h}_{qc}")
                nc.vector.reciprocal_approx_fast(rec, d_ps)
                # normalize per 128-wide row-tile so the wo matmul for tile
                # rt can start as soon as slice rt is written
                for i in range(qc_w // 128):
                    isl = slice(i * 128, (i + 1) * 128)
                    nc.vector.tensor_mul(
                        att_s[:, h, qc * qc_w + i * 128 : qc * qc_w + (i + 1) * 128],
                        o_ps[:, isl], rec[:, isl])
                yield

            def step(g):
                if g is None:
                    return False
                try:
                    next(g)
                    return True
                except StopIteration:
                    return False

            def drain(g):
                while step(g):
                    pass

            def wo_units(qc, rts, pools=None, fine=False):
                """Partial-output block for chunk qc, row-tiles rts: for each
                128-row tile, out[rows, :] = sum_h att_s[:,h,rows].T @ wo_sb[h].
                Pure PE + psum-copy; pss+pso (free whenever no attention
                generator is live) give a 4-bank rotation so the copies
                hide. Yields after each 512-col group."""
                if pools is None:
                    pools = (pss, pso)
                base = qc * qc_w
                for rt in rts:
                    rsl = slice(base + rt * 128, base + (rt + 1) * 128)
                    ost = outp.tile([128, d], F16, tag="osb", name=f"osb{qc}_{rt}")
                    for nk in range(d // 512):
                        nsl = slice(nk * 512, (nk + 1) * 512)
                        pool = pools[nk % len(pools)]
                        w_ps = pool.tile([128, 512], F32, tag=pool.name,
                                         name=f"wps{qc}_{rt}_{nk}")
                        for hh in range(HL):
                            nc.tensor.matmul(
                                w_ps,
                                lhsT=att_s[:, hh, rsl],
                                rhs=wo_sb[:, hh, nsl],
                                start=(hh == 0),
                                stop=(hh == HL - 1),
                            )
                        if fine and rt == rts[-1]:
                            # final tile: quarter-width copies split across
                            # both engines shorten the last serial chain
                            for qq in range(2):
                                qsl2 = slice(nk * 512 + qq * 256,
                                             nk * 512 + (qq + 1) * 256)
                                if qq == 0:
                                    nc.vector.tensor_copy(ost[:, qsl2], w_ps[:, qq * 256 : (qq + 1) * 256])
                                else:
                                    nc.scalar.activation(ost[:, qsl2], w_ps[:, qq * 256 : (qq + 1) * 256], AF.Copy)
                        elif nk % 2 == 0:
                            nc.vector.tensor_copy(ost[:, nsl], w_ps)
                        else:
                            nc.scalar.activation(ost[:, nsl], w_ps, AF.Copy)
                        if fine and rt == rts[-1]:
                            # last tile of the kernel: flush per 512 so the
                            # final HBM write is as small as possible
                            nc.sync.dma_start(out[rsl, nsl], ost[:, nsl])
                        elif nk % 2 == 1:
                            # flush per 1024-col half so the final HBM write
                            # drains behind the remaining compute
                            hsl = slice((nk - 1) * 512, (nk + 1) * 512)
                            nc.sync.dma_start(out[rsl, hsl], ost[:, hsl])
                        yield

            def wo_block(qc, rts=None, fine=False):
                drain(wo_units(qc, rts if rts is not None else range(nst),
                               fine=fine))

            def issue_x(sc2, with_w=False):
                lst = []
                for dp in range(ndt // 2):
                    if with_w:
                        nc.sync.dma_start(
                            wqkv_sb[:, 2 * dp : 2 * dp + 2, :],
                            wqkv_p[:, 2 * dp : 2 * dp + 2, :],
                        )
                    t = xch.tile([128, 2, sc_w], F16, tag="xch", name=f"xch{sc2}_{dp}")
                    nc.sync.dma_start(
                        t, xt_p[:, 2 * dp : 2 * dp + 2,
                                sc2 * sc_w : (sc2 + 1) * sc_w])
                    lst.append(t)
                return lst

            # HAM warm-up: ~3.8us of throwaway matmuls during the initial
            # input-DMA wait, so the clock gate is already at 2.4 GHz when
            # the first real matmul's operands land (cold MMs run at 1.2)
            warm_ps = pso.tile([128, 512], F32, tag="pso", name="warm_ps")
            for i in range(14):
                nc.tensor.matmul(warm_ps, lhsT=ones_s, rhs=warm_s,
                                 start=(i == 0), stop=(i == 13))

            # ---------------- main loop with interleaved attention ----------------
            xnext = None
            for sc in range(nsc):
                scs = slice(sc * sc_w, (sc + 1) * sc_w)
                if sc == 0:
                    xps = issue_x(0, with_w=True)
                    # rope tables + mask + next x chunk + wo, ordered by
                    # first-use time so nothing steals early HBM bandwidth
                    # from the sc0 x/weight stream
                    nc.sync.dma_start(cq_s, cq)
                    if not shared:
                        nc.sync.dma_start(sq_s, sq)
                    nc.sync.dma_start(ck_s, cksk)
                    if causal:
                        nc.sync.dma_start(em_s, emb)
                    xnext = issue_x(1)
                    for hh in range(HL):
                        nc.sync.dma_start(wo_sb[:, hh, :], wo_r[hh])
                    pairs = [(cq_s, cqw_s), (ck_s, ckw_s)]
                    if not shared:
                        pairs.insert(1, (sq_s, sqw_s))
                    for src_t, dst_t in pairs:
                        nc.vector.tensor_copy(dst_t[0:64, :], src_t[64:128, :])
                        nc.vector.tensor_copy(dst_t[64:128, :], src_t[0:64, :])
                else:
                    xps = xnext
                    if sc + 1 < nsc:
                        xnext = issue_x(sc + 1)
                xts = [xps[dt // 2][:, dt % 2, :] for dt in range(ndt)]

                # wo for chunk sc-1 (both heads' attention drained at the end
                # of iteration sc-1) rides this sc's q-chain as pure-PE
                # filler; its psum groups use the pss/pso banks the previous
                # iteration's generators have released by the time the ring
                # comes around
                gwo = wo_units(sc - 1, range(nst)) if (causal and sc >= 1) else None
                q_ps = [psqk.tile([128, sc_w], F32, tag="psqk", name=f"qps{sc}_{_h}")
                        for _h in range(HL)]
                # sc=0 is DMA-bound: fold the k chains into the same per-dt
                # loop (borrowing the pss banks, free before the first
                # attention unit) so the PE does 4 MMs per arriving tile
                # pair instead of 2 and the post-DMA phase starts earlier
                k_pss = None
                if sc == 0:
                    k_pss = [pss.tile([128, sc_w], F32, tag="pss", name=f"kps0_{_h}")
                             for _h in range(HL)]
                for dt in range(ndt):
                    fl = dict(start=(dt == 0), stop=(dt == ndt - 1))
                    for h in range(HL):
                        nc.tensor.matmul(
                            q_ps[h],
                            lhsT=wqkv_sb[:, dt, HD * h : HD * (h + 1)],
                            rhs=xts[dt],
                            **fl,
                        )
                    if k_pss is not None:
                        for h in range(HL):
                            nc.tensor.matmul(
                                k_pss[h],
                                lhsT=wqkv_sb[:, dt, CW + HD * h : CW + HD * (h + 1)],
                                rhs=xts[dt],
                                **fl,
                            )
                    # start the wo filler a few dt-steps in: its psum ring
                    # slots free up only once the previous iteration's
                    # finalization chain clears the DVE queue
                    if dt >= 6:
                        step(gwo)
                _rope_drain(nc, rtmp, q_ps[0], qT_s[:, 0, scs],
                            cq_s[:, scs], cqw_s[:, scs], sc_w)
                _rope_drain(nc, rtmp, q_ps[1], qT_s[:, 1, scs],
                            sq_s[:, scs], sqw_s[:, scs], sc_w)
                drain(gwo)

                # BOTH heads' attention for THIS q chunk: head-h off-band
                # tiles ride the k-h chain, diagonal-band tiles ride the v
                # chains; everything drains inside this iteration
                gens = [attn_units(h, sc) if causal else None for h in range(HL)]
                off_band = nst * sc
                emitted = [0, 0]
                # k0 chain, then k1 chain (each reuses the psum slot of the
                # same-index q_ps, freed by its rope -- the wo filler above
                # guarantees the ropes are long done, so no PE stall)
                k1_ps = None
                if k_pss is not None:
                    _rope_drain(nc, rtmp, k_pss[0], kT_s[:, 0, scs],
                                ck_s[:, scs], ckw_s[:, scs], sc_w)
                    k1_ps = k_pss[1]
                else:
                    for h in range(HL):
                        k_ps = psqk.tile([128, sc_w], F32, tag="psqk",
                                         name=f"kps{sc}_{h}")
                        for dt in range(ndt):
                            nc.tensor.matmul(
                                k_ps,
                                lhsT=wqkv_sb[:, dt, CW + HD * h : CW + HD * (h + 1)],
                                rhs=xts[dt],
                                start=(dt == 0),
                                stop=(dt == ndt - 1),
                            )
                            if dt >= 2 and emitted[h] < off_band:
                                if step(gens[h]):
                                    emitted[h] += 1
                        if h == 0:
                            _rope_drain(nc, rtmp, k_ps, kT_s[:, 0, scs],
                                        ck_s[:, scs], ckw_s[:, scs], sc_w)
                        else:
                            k1_ps = k_ps
                # v chains; rope_k1 + v copies interleave on DVE; both heads'
                # diagonal-band attention tiles follow their v copies
                for st in range(nst):
                    v_ps = psv.tile([128, CW], F32, tag="psv", name=f"vps{sc}_{st}")
                    for dt in range(ndt):
                        nc.tensor.matmul(
                            v_ps,
                            lhsT=xts[dt][:, st * 128 : (st + 1) * 128],
                            rhs=wqkv_sb[:, dt, 2 * CW : 3 * CW],
                            start=(dt == 0),
                            stop=(dt == ndt - 1),
                        )
                    nc.scalar.activation(v_s[:, sc * nst + st, :], v_ps, AF.Copy)
                    if st == 0:
                        _rope_drain(nc, rtmp, k1_ps, kT_s[:, 1, scs],
                                    ck_s[:, scs], ckw_s[:, scs], sc_w)
                    if st >= 1:
                        step(gens[0])
                        step(gens[1])
                drain(gens[0])
                drain(gens[1])

            if causal:
                wo_block(nsc - 1, fine=True)
            else:
                for qc in range(nsc):
                    drain(attn_units(0, qc))
                    drain(attn_units(1, qc))
                    wo_block(qc, fine=(qc == nsc - 1))

    nc.compile()
    return nc


def host_prep(inputs, s=S, d=D, qc_w=512):
    f16 = np.float16
    x = np.ascontiguousarray(np.asarray(inputs["x"], dtype=np.float32)[0])
    wq = np.asarray(inputs["wq"], dtype=np.float32)
    wk = np.asarray(inputs["wk"], dtype=np.float32)
    wv = np.asarray(inputs["wv"], dtype=np.float32)
    wo = np.asarray(inputs["wo"], dtype=np.float32)
    ss = np.asarray(inputs["seq_scale"], dtype=np.float32).reshape(H)
    cos = np.asarray(inputs["freqs_cos"], dtype=np.float32)
    sin = np.asarray(inputs["freqs_sin"], dtype=np.float32)
    mask = np.asarray(inputs["mask"], dtype=np.float32)[0, 0]
    sll = np.asarray(inputs["section_log_len"], dtype=np.float32).reshape(s)

    nst = qc_w // 128
    zero = mask == 0.0
    causal = bool(
        np.array_equal(zero, np.tril(np.ones((s, s), bool)))
        and np.all(mask[~zero] <= -1e8)
    )
    shared = bool(np.all(ss == ss[0]))

    if causal:
        # staircase base [128, qc_w + nst*128]: diagonal pattern m (kpos
        # tile kt = nst*qc + m vs q chunk qc) is emb[:, qc_w-128m :
        # 2*qc_w-128m], i.e. emb[dk, c] = 1 iff (dk + 128m) <= dq with
        # dq = c - (qc_w - 128m)  <=>  dk <= c - qc_w
        w_em = qc_w + nst * 128
        ccol = np.arange(w_em)[None, :]
        crow = np.arange(128)[:, None]
        em_in = np.ascontiguousarray(
            (crow <= ccol - qc_w).astype(np.float32).astype(bf16)
        )
    else:
        em_in = np.ascontiguousarray(np.exp(np.minimum(mask, 0.0)).T.astype(bf16))

    perm = np.concatenate([np.arange(0, HD, 2), np.arange(1, HD, 2)])
    xt = np.ascontiguousarray(x.T.astype(f16))
    scale = sll / np.sqrt(HD)
    cksk = np.ascontiguousarray(np.concatenate([cos.T, sin.T], axis=0).astype(f16))

    in_maps = []
    for i in range(NC):
        wq_s = np.concatenate(
            [wq[:, CW * i + HD * h : CW * i + HD * (h + 1)][:, perm] for h in range(HL)],
            axis=1,
        )
        wk_s = np.concatenate(
            [wk[:, CW * i + HD * h : CW * i + HD * (h + 1)][:, perm] for h in range(HL)],
            axis=1,
        )
        wv_s = wv[:, CW * i : CW * (i + 1)]
        # per-head packed [cos; sin] scaled tables (one shared table when
        # seq_scale is uniform)
        cqt = np.concatenate(
            [cos.T * (scale * ss[HL * i])[None, :],
             sin.T * (scale * ss[HL * i])[None, :]], axis=0
        )
        m = {
            "xt": xt,
            "wqkv": np.ascontiguousarray(
                np.concatenate([wq_s, wk_s, wv_s], axis=1).astype(f16)
            ),
            "wo": np.ascontiguousarray(wo[CW * i : CW * (i + 1)].astype(bf16)),
            "cq": np.ascontiguousarray(cqt.astype(f16)),
            "cksk": cksk,
            "em": em_in,
        }
        if not shared:
            sqt = np.concatenate(
                [cos.T * (scale * ss[HL * i + 1])[None, :],
                 sin.T * (scale * ss[HL * i + 1])[None, :]], axis=0
            )
            m["sq"] = np.ascontiguousarray(sqt.astype(f16))
        in_maps.append(m)
    return in_maps, causal, shared


_NC_CACHE = {}


def _get_nc(causal, shared):
    key = (causal, shared)
    if key not in _NC_CACHE:
        _NC_CACHE[key] = build_nc(causal, shared)
    return _NC_CACHE[key]


def gather(res):
    """Sum the 8 fp16 partial outputs (unshard for a partial-sum layout)."""
    acc = res.results[0]["out"].astype(np.float32)
    for i in range(1, NC):
        acc += res.results[i]["out"].astype(np.float32)
    return acc[None]


def kernel(**inputs) -> np.ndarray:
    in_maps, causal, shared = host_prep(inputs)
    nc = _get_nc(causal, shared)
    res = run_bass_kernel_spmd(nc, in_maps, core_ids=list(range(NC)))
    return gather(res)
